# revision 6
# baseline (speedup 1.0000x reference)
"""Separable depthwise box filter (r=8, 'same' zero padding) on 8 trn2 cores.

Math: per (n, c) plane P (512x512), out = s^2 * (Bo @ P @ Bo) where Bo is the
symmetric banded 512x512 matrix of ONES with |i - j| <= r and s = 1/(2r+1).
Computing with a band of ones keeps B exact in bf16; the s^2 normalization is
folded into the pass-2 PSUM evacuation copy (fp32 scale, then bf16 cast).

On the PE (out = lhsT.T @ rhs):

  pass 1: Zt = matmul(lhsT=P,  rhs=Bo) = P.T @ Bo  (vertical filter, transposed)
  pass 2: Y  = matmul(lhsT=Zt, rhs=Bo) = Z  @ Bo   (horizontal filter, restored)

Everything on-chip is bf16 (inputs cast on host): fp32 matmuls run as HI/LO
pairs at 2x stream cost and disable fast weight load, so bf16 halves PE time
AND halves HBM traffic.  PSUM accumulates in fp32, so only the band sums --
not the accumulation -- see bf16 rounding.

Both passes stream only the banded columns of Bo: the K-chunk of rows
[128a, 128a+128) of Bo has nonzero columns only in [128a-r, 128a+128+r).
PSUM's per-element has_written bit makes the overlapping column windows
accumulate while fresh columns overwrite, so each (M-chunk, K-chunk) pair is
a single matmul: 560 streamed columns per M-chunk instead of 2048.

Sharding: batch dim (8) across the 8 cores; each core filters its 16 channel
planes independently (no cross-core communication).
"""

import numpy as np

_CACHE = {}

N_CORES = 8
P = 128
H = W = 512
A = H // P  # 4 row-chunks per plane


def _band_windows(r):
    """Nonzero column window [n0, n1) of Bo rows [128a, 128a+128), per a."""
    return [(max(0, P * a - r), min(W, P * a + P + r)) for a in range(A)]


def _build(r, n_planes):
    import concourse.mybir as mybir
    from concourse import bacc
    from concourse.tile import TileContext

    bf16 = mybir.dt.bfloat16
    f32 = mybir.dt.float32
    win = _band_windows(r)
    inv_k2 = float(1.0 / float(2 * r + 1) ** 2)

    nc = bacc.Bacc()
    x_d = nc.declare_dram_parameter("x", [n_planes * H, W], bf16, isOutput=False)
    b_d = nc.declare_dram_parameter("b", [H, W], bf16, isOutput=False)
    y_d = nc.declare_dram_parameter("y", [n_planes * H, W], bf16, isOutput=True)

    x_ap = x_d.ap().rearrange("(p a q) n -> p q a n", p=n_planes, q=P)
    y_ap = y_d.ap().rearrange("(p a q) n -> p q a n", p=n_planes, q=P)
    b_ap = b_d.ap().rearrange("(a q) n -> q a n", q=P)

    with TileContext(nc) as tc:
        with (
            tc.tile_pool(name="bmat", bufs=1) as bpool,
            tc.tile_pool(name="xin", bufs=3) as xpool,
            tc.tile_pool(name="zmid", bufs=2) as zpool,
            tc.tile_pool(name="yout", bufs=3) as opool,
            tc.tile_pool(name="ps", bufs=4, space="PSUM") as psp,
        ):
            bt = bpool.tile([P, A, W], bf16)
            xt0 = xpool.tile([P, A, W], bf16, name="xt0", tag="xt")
            # Interleave plane-0 chunks with Bo chunks so the first matmul can
            # start after ~170 KiB instead of the full ~1 MiB preload.
            w0, w1 = win[0]
            nc.sync.dma_start(out=bt[:, 0, w0:w1], in_=b_ap[:, 0, w0:w1])
            nc.sync.dma_start(out=xt0[:, 0, :], in_=x_ap[0, :, 0, :])
            for a in range(1, A):
                nc.sync.dma_start(out=bt[:, a, :], in_=b_ap[:, a, :])
                nc.sync.dma_start(out=xt0[:, a, :], in_=x_ap[0, :, a, :])
            nc.sync.dma_start(out=bt[:, 0, w1:W], in_=b_ap[:, 0, w1:W])

            for p in range(n_planes):
                if p == 0:
                    xt = xt0
                else:
                    xt = xpool.tile([P, A, W], bf16, name="xt", tag="xt")
                    nc.sync.dma_start(out=xt[:], in_=x_ap[p])

                # PSUM tiles are bank PAIRS: one evac op per 2 banks halves
                # the fixed per-op cost and semaphore traffic on DVE/ACT.
                zt = zpool.tile([P, A, W], bf16)
                for half in range(2):
                    ps = psp.tile([P, 2, W], f32, name="ps1", tag="ps")
                    for mm in range(2):
                        m = 2 * half + mm
                        for a in range(A):
                            n0, n1 = win[a]
                            nc.tensor.matmul(
                                ps[:, mm, n0:n1],
                                xt[:, a, m * P : (m + 1) * P],
                                bt[:, a, n0:n1],
                                start=(a == 0),
                                stop=(a == A - 1),
                                skip_group_check=True,
                            )
                    if half == 0:
                        nc.vector.tensor_copy(out=zt[:, 0:2, :], in_=ps[:])
                    else:
                        nc.scalar.copy(out=zt[:, 2:4, :], in_=ps[:])

                ot = opool.tile([P, A, W], bf16)
                for half in range(2):
                    ps = psp.tile([P, 2, W], f32, name="ps2", tag="ps")
                    for mm in range(2):
                        m = 2 * half + mm
                        for a in range(A):
                            n0, n1 = win[a]
                            nc.tensor.matmul(
                                ps[:, mm, n0:n1],
                                zt[:, a, m * P : (m + 1) * P],
                                bt[:, a, n0:n1],
                                start=(a == 0),
                                stop=(a == A - 1),
                                skip_group_check=True,
                            )
                    # pass-2 evac folds in the 1/(2r+1)^2 normalization;
                    # engine alternation mirrors pass 1 to balance DVE/ACT
                    if half == 0:
                        nc.scalar.mul(ot[:, 0:2, :], ps[:], inv_k2)
                    else:
                        nc.vector.tensor_scalar_mul(ot[:, 2:4, :], ps[:], inv_k2)
                    # store each half-plane via the GpSimd SWDGE queue: a
                    # third DMA path so input (SP ring) and output flow on
                    # separate queues without costing DVE/ACT engine time
                    nc.gpsimd.dma_start(
                        out=y_ap[p, :, 2 * half : 2 * half + 2, :],
                        in_=ot[:, 2 * half : 2 * half + 2, :],
                    )

    # Drop the preamble's GpSimd memsets of unused const tiles: Q7 memsets
    # cost ~µs each and gate the post-preamble all-engine barrier, delaying
    # kernel start.  Keep any const a later instruction actually reads.
    used = set()
    for bb in nc.main_func.blocks:
        for inst in bb.instructions:
            if type(inst).__name__ == "InstMemset":
                continue
            for ap in list(inst.ins or []) + list(inst.outs or []):
                ref = getattr(ap, "memref", None)
                if ref and str(ref).startswith("const-"):
                    used.add(str(ref))
    entry = nc.main_func.blocks[0]
    dropped = [
        inst
        for inst in entry.instructions
        if type(inst).__name__ == "InstMemset"
        and inst.outs
        and str(getattr(inst.outs[0], "memref", "")).startswith("const-")
        and str(inst.outs[0].memref) not in used
    ]
    for inst in dropped:
        entry.instructions.remove(inst)

    nc.finalize()
    return nc


def _band_ones(r):
    b = np.zeros((H, W), dtype=np.float32)
    for i in range(H):
        b[i, max(0, i - r) : min(W, i + r + 1)] = 1.0
    return b


def kernel(x, r):
    import ml_dtypes
    from concourse.bass_utils import run_bass_kernel_spmd

    r = int(r)
    x = np.asarray(x)
    n, c, h, w = x.shape
    assert (h, w) == (H, W) and n == N_CORES, (n, c, h, w)

    key = (r, c)
    if key not in _CACHE:
        _CACHE[key] = _build(r, c)
    nc = _CACHE[key]

    bf16 = ml_dtypes.bfloat16
    xb = np.ascontiguousarray(x.reshape(n, c * H, W)).astype(bf16)
    b = _band_ones(r).astype(bf16)
    in_maps = [{"x": xb[i], "b": b} for i in range(n)]
    res = run_bass_kernel_spmd(nc, in_maps, core_ids=list(range(N_CORES)))
    out = np.stack(
        [np.asarray(res.results[i]["y"]).astype(np.float32).reshape(c, H, W) for i in range(n)]
    )
    return out


# revision 8
# speedup vs baseline: 1.1250x; 1.1250x over previous
"""Separable depthwise box filter (r=8, 'same' zero padding) on 8 trn2 cores.

Math: per (n, c) plane P (512x512), out = s^2 * (Bo @ P @ Bo) where Bo is the
symmetric banded 512x512 matrix of ONES with |i - j| <= r and s = 1/(2r+1).
Computing with a band of ones keeps Bo exact in bf16; the s^2 normalization is
folded into the pass-2 PSUM evacuation (fp32 scale, then bf16 cast).

On the PE (out = lhsT.T @ rhs):

  pass 1: Zt = matmul(lhsT=P,  rhs=Bo) = P.T @ Bo  (vertical filter, transposed)
  pass 2: Y  = matmul(lhsT=Zt, rhs=Bo) = Z  @ Bo   (horizontal filter, restored)

Everything on-chip is bf16 (inputs cast on host): fp32 matmuls run as HI/LO
pairs at 2x stream cost, so bf16 halves PE time AND halves HBM traffic.  PSUM
accumulates in fp32, so only the band sums see bf16 rounding.

Only the banded column windows of Bo are streamed AND loaded: the K-chunk of
rows [128a, 128a+128) of Bo has nonzero columns only in [128a-r, 128a+128+r).
PSUM's per-element has_written bit makes the overlapping column windows
accumulate while fresh columns overwrite.

Pipeline structure (per core, 16 planes):
  - planes are software-pipelined: pass1(p+1) is emitted between pass1(p) and
    pass2(p), so the PE fills the Z-evacuation latency with the next plane's
    pass-1 matmuls instead of stalling (keeps HAM un-throttled, 2.4 GHz).
  - PSUM tiles are bank PAIRS ([128, 2, 512] f32): one evac op per 2 banks
    halves the fixed per-op cost and semaphore traffic on DVE/ACT.
  - evacs alternate DVE/ACT; input loads ride the SP HWDGE ring, output
    stores the GpSimd SWDGE queue, so in/out flow on separate DMA queues and
    cost no DVE/ACT engine time.

Sharding: batch dim (8) across the 8 cores; each core filters its 16 channel
planes independently (no cross-core communication).
"""

import numpy as np

_CACHE = {}

N_CORES = 8
P = 128
H = W = 512
A = H // P  # 4 row-chunks per plane


def _band_windows(r):
    """Nonzero column window [n0, n1) of Bo rows [128a, 128a+128), per a."""
    return [(max(0, P * a - r), min(W, P * a + P + r)) for a in range(A)]


def _build(r, n_planes):
    import concourse.mybir as mybir
    from concourse import bacc
    from concourse.tile import TileContext

    bf16 = mybir.dt.bfloat16
    f32 = mybir.dt.float32
    win = _band_windows(r)
    wmax = max(n1 - n0 for n0, n1 in win)
    inv_k2 = float(1.0 / float(2 * r + 1) ** 2)

    nc = bacc.Bacc()
    x_d = nc.declare_dram_parameter("x", [n_planes * H, W], bf16, isOutput=False)
    # b holds only the banded windows, packed: row-chunk a's window in b[:, a, :]
    b_d = nc.declare_dram_parameter("b", [P, A * wmax], bf16, isOutput=False)
    y_d = nc.declare_dram_parameter("y", [n_planes * H, W], bf16, isOutput=True)

    x_ap = x_d.ap().rearrange("(p a q) n -> p q a n", p=n_planes, q=P)
    y_ap = y_d.ap().rearrange("(p a q) n -> p q a n", p=n_planes, q=P)
    b_ap = b_d.ap().rearrange("q (a n) -> q a n", a=A)

    with TileContext(nc) as tc:
        with (
            tc.tile_pool(name="bmat", bufs=1) as bpool,
            tc.tile_pool(name="xin", bufs=3) as xpool,
            tc.tile_pool(name="zmid", bufs=2) as zpool,
            tc.tile_pool(name="yout", bufs=3) as opool,
            tc.tile_pool(name="ps", bufs=4, space="PSUM") as psp,
        ):
            bt = bpool.tile([P, A, wmax], bf16)
            xt0 = xpool.tile([P, A, W], bf16, name="xt0", tag="xt")
            # Interleave plane-0 chunks with Bo windows so the first matmul
            # can start after ~160 KiB instead of a full preload.
            nc.sync.dma_start(out=bt[:, 0, :], in_=b_ap[:, 0, :])
            nc.sync.dma_start(out=xt0[:, 0, :], in_=x_ap[0, :, 0, :])
            for a in range(1, A):
                nc.sync.dma_start(out=bt[:, a, :], in_=b_ap[:, a, :])
                nc.sync.dma_start(out=xt0[:, a, :], in_=x_ap[0, :, a, :])

            xts = {0: xt0}
            zts = {}

            def pass1(p):
                if p not in xts:
                    xt = xpool.tile([P, A, W], bf16, name="xt", tag="xt")
                    nc.sync.dma_start(out=xt[:], in_=x_ap[p])
                    xts[p] = xt
                xt = xts[p]
                zt = zpool.tile([P, A, W], bf16)
                zts[p] = zt
                for half in range(2):
                    ps = psp.tile([P, 2, W], f32, name="ps1", tag="ps")
                    for mm in range(2):
                        m = 2 * half + mm
                        for a in range(A):
                            n0, n1 = win[a]
                            nc.tensor.matmul(
                                ps[:, mm, n0:n1],
                                xt[:, a, m * P : (m + 1) * P],
                                bt[:, a, : n1 - n0],
                                start=(a == 0),
                                stop=(a == A - 1),
                                skip_group_check=True,
                            )
                    if half == 0:
                        nc.vector.tensor_copy(out=zt[:, 0:2, :], in_=ps[:])
                    else:
                        nc.scalar.copy(out=zt[:, 2:4, :], in_=ps[:])
                del xts[p]

            def pass2(p):
                zt = zts.pop(p)
                ot = opool.tile([P, A, W], bf16)
                for half in range(2):
                    ps = psp.tile([P, 2, W], f32, name="ps2", tag="ps")
                    for mm in range(2):
                        m = 2 * half + mm
                        for a in range(A):
                            n0, n1 = win[a]
                            nc.tensor.matmul(
                                ps[:, mm, n0:n1],
                                zt[:, a, m * P : (m + 1) * P],
                                bt[:, a, : n1 - n0],
                                start=(a == 0),
                                stop=(a == A - 1),
                                skip_group_check=True,
                            )
                    # pass-2 evac folds in the 1/(2r+1)^2 normalization;
                    # engine alternation mirrors pass 1 to balance DVE/ACT
                    if half == 0:
                        nc.scalar.mul(ot[:, 0:2, :], ps[:], inv_k2)
                    else:
                        nc.vector.tensor_scalar_mul(ot[:, 2:4, :], ps[:], inv_k2)
                    nc.gpsimd.dma_start(
                        out=y_ap[p, :, 2 * half : 2 * half + 2, :],
                        in_=ot[:, 2 * half : 2 * half + 2, :],
                    )

            pass1(0)
            for p in range(n_planes):
                if p + 1 < n_planes:
                    pass1(p + 1)
                pass2(p)

    # Drop the preamble's GpSimd memsets of unused const tiles: Q7 memsets
    # cost ~us each and gate the post-preamble all-engine barrier, delaying
    # kernel start.  Keep any const a later instruction actually reads.
    used = set()
    for bb in nc.main_func.blocks:
        for inst in bb.instructions:
            if type(inst).__name__ == "InstMemset":
                continue
            for ap in list(inst.ins or []) + list(inst.outs or []):
                ref = getattr(ap, "memref", None)
                if ref and str(ref).startswith("const-"):
                    used.add(str(ref))
    entry = nc.main_func.blocks[0]
    dropped = [
        inst
        for inst in entry.instructions
        if type(inst).__name__ == "InstMemset"
        and inst.outs
        and str(getattr(inst.outs[0], "memref", "")).startswith("const-")
        and str(inst.outs[0].memref) not in used
    ]
    for inst in dropped:
        entry.instructions.remove(inst)

    nc.finalize()
    return nc


def _band_windows_payload(r):
    """Packed nonzero windows of the band-of-ones matrix, [P, A*wmax] f32."""
    win = _band_windows(r)
    wmax = max(n1 - n0 for n0, n1 in win)
    out = np.zeros((P, A * wmax), dtype=np.float32)
    for a, (n0, n1) in enumerate(win):
        for q in range(P):
            i = P * a + q  # absolute row of Bo
            lo = max(n0, i - r, 0)
            hi = min(n1, i + r + 1, W)
            out[q, a * wmax + (lo - n0) : a * wmax + (hi - n0)] = 1.0
    return out


def kernel(x, r):
    import ml_dtypes
    from concourse.bass_utils import run_bass_kernel_spmd

    r = int(r)
    x = np.asarray(x)
    n, c, h, w = x.shape
    assert (h, w) == (H, W) and n == N_CORES, (n, c, h, w)

    key = (r, c)
    if key not in _CACHE:
        _CACHE[key] = _build(r, c)
    nc = _CACHE[key]

    bf16 = ml_dtypes.bfloat16
    xb = np.ascontiguousarray(x.reshape(n, c * H, W)).astype(bf16)
    b = _band_windows_payload(r).astype(bf16)
    in_maps = [{"x": xb[i], "b": b} for i in range(n)]
    res = run_bass_kernel_spmd(nc, in_maps, core_ids=list(range(N_CORES)))
    out = np.stack(
        [np.asarray(res.results[i]["y"]).astype(np.float32).reshape(c, H, W) for i in range(n)]
    )
    return out


# revision 10
# speedup vs baseline: 1.1814x; 1.0501x over previous
"""Separable depthwise box filter (r=8, 'same' zero padding) on 8 trn2 cores.

Math: per (n, c) plane P (512x512), out = s^2 * (Bo @ P @ Bo) where Bo is the
symmetric banded 512x512 matrix of ONES with |i - j| <= r and s = 1/(2r+1).
Computing with a band of ones keeps Bo exact in bf16; the s^2 normalization is
folded into the pass-2 PSUM evacuation (fp32 scale, then bf16 cast).

On the PE (out = lhsT.T @ rhs):

  pass 1: Zt = matmul(lhsT=P,  rhs=Bo) = P.T @ Bo  (vertical filter, transposed)
  pass 2: Y  = matmul(lhsT=Zt, rhs=Bo) = Z  @ Bo   (horizontal filter, restored)

Everything on-chip is bf16 (inputs cast on host): fp32 matmuls run as HI/LO
pairs at 2x stream cost, so bf16 halves PE time AND halves HBM traffic.  PSUM
accumulates in fp32, so only the band sums see bf16 rounding.

Only the banded column windows of Bo are streamed AND loaded: the K-chunk of
rows [128a, 128a+128) of Bo has nonzero columns only in [128a-r, 128a+128+r).
PSUM's per-element has_written bit makes the overlapping column windows
accumulate while fresh columns overwrite.

Pipeline structure (per core, 16 planes):
  - planes are software-pipelined: pass1(p+1) is emitted between pass1(p) and
    pass2(p), so the PE fills the Z-evacuation latency with the next plane's
    pass-1 matmuls instead of stalling (keeps HAM un-throttled, 2.4 GHz).
  - PSUM tiles are bank PAIRS ([128, 2, 512] f32): one evac op per 2 banks
    halves the fixed per-op cost and semaphore traffic on DVE/ACT.
  - evacs alternate DVE/ACT; input loads ride the SP HWDGE ring, output
    stores the GpSimd SWDGE queue, so in/out flow on separate DMA queues and
    cost no DVE/ACT engine time.

Sharding: batch dim (8) across the 8 cores; each core filters its 16 channel
planes independently (no cross-core communication).
"""

import numpy as np

_CACHE = {}

N_CORES = 8
P = 128
H = W = 512
A = H // P  # 4 row-chunks per plane


def _band_windows(r):
    """Nonzero column window [n0, n1) of Bo rows [128a, 128a+128), per a."""
    return [(max(0, P * a - r), min(W, P * a + P + r)) for a in range(A)]


def _build(r, n_planes):
    import concourse.mybir as mybir
    from concourse import bacc
    from concourse.tile import TileContext

    bf16 = mybir.dt.bfloat16
    f32 = mybir.dt.float32
    win = _band_windows(r)
    wmax = max(n1 - n0 for n0, n1 in win)
    inv_k2 = float(1.0 / float(2 * r + 1) ** 2)

    nc = bacc.Bacc()
    x_d = nc.declare_dram_parameter("x", [n_planes * H, W], bf16, isOutput=False)
    # b holds only the banded windows, packed: row-chunk a's window in b[:, a, :]
    b_d = nc.declare_dram_parameter("b", [P, A * wmax], bf16, isOutput=False)
    y_d = nc.declare_dram_parameter("y", [n_planes * H, W], bf16, isOutput=True)

    x_ap = x_d.ap().rearrange("(p a q) n -> p q a n", p=n_planes, q=P)
    y_ap = y_d.ap().rearrange("(p a q) n -> p q a n", p=n_planes, q=P)
    b_ap = b_d.ap().rearrange("q (a n) -> q a n", a=A)

    with TileContext(nc) as tc:
        with (
            tc.tile_pool(name="bmat", bufs=1) as bpool,
            tc.tile_pool(name="xin", bufs=5) as xpool,
            tc.tile_pool(name="zmid", bufs=3) as zpool,
            tc.tile_pool(name="yout", bufs=4) as opool,
            tc.tile_pool(name="ps", bufs=4, space="PSUM") as psp,
        ):
            bt = bpool.tile([P, A, wmax], bf16)
            xt0 = xpool.tile([P, A, W], bf16, name="xt0", tag="xt")
            # Interleave plane-0 chunks with Bo windows so the first matmul
            # can start after ~160 KiB instead of a full preload.
            nc.sync.dma_start(out=bt[:, 0, :], in_=b_ap[:, 0, :])
            nc.sync.dma_start(out=xt0[:, 0, :], in_=x_ap[0, :, 0, :])
            for a in range(1, A):
                nc.sync.dma_start(out=bt[:, a, :], in_=b_ap[:, a, :])
                nc.sync.dma_start(out=xt0[:, a, :], in_=x_ap[0, :, a, :])

            xts = {0: xt0}
            zts = {}

            def pass1(p):
                if p not in xts:
                    xt = xpool.tile([P, A, W], bf16, name="xt", tag="xt")
                    nc.sync.dma_start(out=xt[:], in_=x_ap[p])
                    xts[p] = xt
                xt = xts[p]
                zt = zpool.tile([P, A, W], bf16)
                zts[p] = zt
                for half in range(2):
                    ps = psp.tile([P, 2, W], f32, name="ps1", tag="ps")
                    for mm in range(2):
                        m = 2 * half + mm
                        for a in range(A):
                            n0, n1 = win[a]
                            nc.tensor.matmul(
                                ps[:, mm, n0:n1],
                                xt[:, a, m * P : (m + 1) * P],
                                bt[:, a, : n1 - n0],
                                start=(a == 0),
                                stop=(a == A - 1),
                                skip_group_check=True,
                            )
                    if half == 0:
                        nc.vector.tensor_copy(out=zt[:, 0:2, :], in_=ps[:])
                    else:
                        nc.scalar.copy(out=zt[:, 2:4, :], in_=ps[:])
                del xts[p]

            def pass2(p):
                zt = zts.pop(p)
                ot = opool.tile([P, A, W], bf16)
                for half in range(2):
                    ps = psp.tile([P, 2, W], f32, name="ps2", tag="ps")
                    for mm in range(2):
                        m = 2 * half + mm
                        for a in range(A):
                            n0, n1 = win[a]
                            nc.tensor.matmul(
                                ps[:, mm, n0:n1],
                                zt[:, a, m * P : (m + 1) * P],
                                bt[:, a, : n1 - n0],
                                start=(a == 0),
                                stop=(a == A - 1),
                                skip_group_check=True,
                            )
                    # pass-2 evac folds in the 1/(2r+1)^2 normalization;
                    # engine alternation mirrors pass 1 to balance DVE/ACT
                    if half == 0:
                        nc.scalar.mul(ot[:, 0:2, :], ps[:], inv_k2)
                    else:
                        nc.vector.tensor_scalar_mul(ot[:, 2:4, :], ps[:], inv_k2)
                    nc.gpsimd.dma_start(
                        out=y_ap[p, :, 2 * half : 2 * half + 2, :],
                        in_=ot[:, 2 * half : 2 * half + 2, :],
                    )

            # depth-3 software pipeline: pass1 runs TWO planes ahead of pass2,
            # so pass2(p)'s zt dependency has a full plane of slack and the
            # PE never stalls on the pass-1 evacuations.
            pass1(0)
            pass1(1)
            for p in range(n_planes):
                if p + 2 < n_planes:
                    pass1(p + 2)
                pass2(p)

    # Drop the preamble's GpSimd memsets of unused const tiles: Q7 memsets
    # cost ~us each and gate the post-preamble all-engine barrier, delaying
    # kernel start.  Keep any const a later instruction actually reads.
    used = set()
    for bb in nc.main_func.blocks:
        for inst in bb.instructions:
            if type(inst).__name__ == "InstMemset":
                continue
            for ap in list(inst.ins or []) + list(inst.outs or []):
                ref = getattr(ap, "memref", None)
                if ref and str(ref).startswith("const-"):
                    used.add(str(ref))
    entry = nc.main_func.blocks[0]
    dropped = [
        inst
        for inst in entry.instructions
        if type(inst).__name__ == "InstMemset"
        and inst.outs
        and str(getattr(inst.outs[0], "memref", "")).startswith("const-")
        and str(inst.outs[0].memref) not in used
    ]
    for inst in dropped:
        entry.instructions.remove(inst)

    nc.finalize()
    return nc


def _band_windows_payload(r):
    """Packed nonzero windows of the band-of-ones matrix, [P, A*wmax] f32."""
    win = _band_windows(r)
    wmax = max(n1 - n0 for n0, n1 in win)
    out = np.zeros((P, A * wmax), dtype=np.float32)
    for a, (n0, n1) in enumerate(win):
        for q in range(P):
            i = P * a + q  # absolute row of Bo
            lo = max(n0, i - r, 0)
            hi = min(n1, i + r + 1, W)
            out[q, a * wmax + (lo - n0) : a * wmax + (hi - n0)] = 1.0
    return out


def kernel(x, r):
    import ml_dtypes
    from concourse.bass_utils import run_bass_kernel_spmd

    r = int(r)
    x = np.asarray(x)
    n, c, h, w = x.shape
    assert (h, w) == (H, W) and n == N_CORES, (n, c, h, w)

    key = (r, c)
    if key not in _CACHE:
        _CACHE[key] = _build(r, c)
    nc = _CACHE[key]

    bf16 = ml_dtypes.bfloat16
    xb = np.ascontiguousarray(x.reshape(n, c * H, W)).astype(bf16)
    b = _band_windows_payload(r).astype(bf16)
    in_maps = [{"x": xb[i], "b": b} for i in range(n)]
    res = run_bass_kernel_spmd(nc, in_maps, core_ids=list(range(N_CORES)))
    out = np.stack(
        [np.asarray(res.results[i]["y"]).astype(np.float32).reshape(c, H, W) for i in range(n)]
    )
    return out


# revision 11
# speedup vs baseline: 1.1883x; 1.0058x over previous
"""Separable depthwise box filter (r=8, 'same' zero padding) on 8 trn2 cores.

Math: per (n, c) plane P (512x512), out = s^2 * (Bo @ P @ Bo) where Bo is the
symmetric banded 512x512 matrix of ONES with |i - j| <= r and s = 1/(2r+1).
Computing with a band of ones keeps Bo exact in bf16; the s^2 normalization is
folded into the pass-2 PSUM evacuation (fp32 scale, then bf16 cast).

On the PE (out = lhsT.T @ rhs):

  pass 1: Zt = matmul(lhsT=P,  rhs=Bo) = P.T @ Bo  (vertical filter, transposed)
  pass 2: Y  = matmul(lhsT=Zt, rhs=Bo) = Z  @ Bo   (horizontal filter, restored)

Everything on-chip is bf16 (inputs cast on host): fp32 matmuls run as HI/LO
pairs at 2x stream cost, so bf16 halves PE time AND halves HBM traffic.  PSUM
accumulates in fp32, so only the band sums see bf16 rounding.

Only the banded column windows of Bo are streamed AND loaded: the K-chunk of
rows [128a, 128a+128) of Bo has nonzero columns only in [128a-r, 128a+128+r).
PSUM's per-element has_written bit makes the overlapping column windows
accumulate while fresh columns overwrite.

Pipeline structure (per core, 16 planes):
  - planes are software-pipelined: pass1(p+1) is emitted between pass1(p) and
    pass2(p), so the PE fills the Z-evacuation latency with the next plane's
    pass-1 matmuls instead of stalling (keeps HAM un-throttled, 2.4 GHz).
  - PSUM tiles are bank PAIRS ([128, 2, 512] f32): one evac op per 2 banks
    halves the fixed per-op cost and semaphore traffic on DVE/ACT.
  - evacs alternate DVE/ACT; input loads ride the SP HWDGE ring, output
    stores the GpSimd SWDGE queue, so in/out flow on separate DMA queues and
    cost no DVE/ACT engine time.

Sharding: batch dim (8) across the 8 cores; each core filters its 16 channel
planes independently (no cross-core communication).
"""

import numpy as np

_CACHE = {}

N_CORES = 8
P = 128
H = W = 512
A = H // P  # 4 row-chunks per plane


def _band_windows(r):
    """Nonzero column window [n0, n1) of Bo rows [128a, 128a+128), per a."""
    return [(max(0, P * a - r), min(W, P * a + P + r)) for a in range(A)]


def _build(r, n_planes):
    import concourse.mybir as mybir
    from concourse import bacc
    from concourse.tile import TileContext

    bf16 = mybir.dt.bfloat16
    f32 = mybir.dt.float32
    win = _band_windows(r)
    wmax = max(n1 - n0 for n0, n1 in win)
    inv_k2 = float(1.0 / float(2 * r + 1) ** 2)

    nc = bacc.Bacc()
    x_d = nc.declare_dram_parameter("x", [n_planes * H, W], bf16, isOutput=False)
    # b holds only the banded windows, packed: row-chunk a's window in b[:, a, :]
    b_d = nc.declare_dram_parameter("b", [P, A * wmax], bf16, isOutput=False)
    y_d = nc.declare_dram_parameter("y", [n_planes * H, W], bf16, isOutput=True)

    x_ap = x_d.ap().rearrange("(p a q) n -> p q a n", p=n_planes, q=P)
    y_ap = y_d.ap().rearrange("(p a q) n -> p q a n", p=n_planes, q=P)
    b_ap = b_d.ap().rearrange("q (a n) -> q a n", a=A)

    with TileContext(nc) as tc:
        with (
            tc.tile_pool(name="bmat", bufs=1) as bpool,
            tc.tile_pool(name="xin", bufs=5) as xpool,
            tc.tile_pool(name="zmid", bufs=3) as zpool,
            tc.tile_pool(name="yout", bufs=4) as opool,
            tc.tile_pool(name="ps", bufs=4, space="PSUM") as psp,
        ):
            bt = bpool.tile([P, A, wmax], bf16)
            xt0 = xpool.tile([P, A, W], bf16, name="xt0", tag="xt")
            # Interleave plane-0 chunks with Bo windows so the first matmul
            # can start after ~160 KiB instead of a full preload.
            nc.sync.dma_start(out=bt[:, 0, :], in_=b_ap[:, 0, :])
            nc.sync.dma_start(out=xt0[:, 0, :], in_=x_ap[0, :, 0, :])
            for a in range(1, A):
                nc.sync.dma_start(out=bt[:, a, :], in_=b_ap[:, a, :])
                nc.sync.dma_start(out=xt0[:, a, :], in_=x_ap[0, :, a, :])

            xts = {0: xt0}
            zts = {}

            def pass1(p):
                if p not in xts:
                    xt = xpool.tile([P, A, W], bf16, name="xt", tag="xt")
                    nc.sync.dma_start(out=xt[:], in_=x_ap[p])
                    xts[p] = xt
                xt = xts[p]
                zt = zpool.tile([P, A, W], bf16)
                zts[p] = zt
                for half in range(2):
                    ps = psp.tile([P, 2, W], f32, name="ps1", tag="ps")
                    for mm in range(2):
                        m = 2 * half + mm
                        for a in range(A):
                            n0, n1 = win[a]
                            nc.tensor.matmul(
                                ps[:, mm, n0:n1],
                                xt[:, a, m * P : (m + 1) * P],
                                bt[:, a, : n1 - n0],
                                start=(a == 0),
                                stop=(a == A - 1),
                                skip_group_check=True,
                            )
                    # both pass-1 evacs on DVE: each engine's ops then
                    # complete in PE order, avoiding head-of-line sem waits
                    nc.vector.tensor_copy(out=zt[:, 2 * half : 2 * half + 2, :], in_=ps[:])
                del xts[p]

            def pass2(p):
                zt = zts.pop(p)
                ot = opool.tile([P, A, W], bf16)
                for half in range(2):
                    ps = psp.tile([P, 2, W], f32, name="ps2", tag="ps")
                    for mm in range(2):
                        m = 2 * half + mm
                        for a in range(A):
                            n0, n1 = win[a]
                            nc.tensor.matmul(
                                ps[:, mm, n0:n1],
                                zt[:, a, m * P : (m + 1) * P],
                                bt[:, a, : n1 - n0],
                                start=(a == 0),
                                stop=(a == A - 1),
                                skip_group_check=True,
                            )
                    # pass-2 evacs (with the 1/(2r+1)^2 scale) on ACT
                    nc.scalar.mul(ot[:, 2 * half : 2 * half + 2, :], ps[:], inv_k2)
                    nc.gpsimd.dma_start(
                        out=y_ap[p, :, 2 * half : 2 * half + 2, :],
                        in_=ot[:, 2 * half : 2 * half + 2, :],
                    )

            # depth-3 software pipeline: pass1 runs TWO planes ahead of pass2,
            # so pass2(p)'s zt dependency has a full plane of slack and the
            # PE never stalls on the pass-1 evacuations.
            pass1(0)
            pass1(1)
            for p in range(n_planes):
                if p + 2 < n_planes:
                    pass1(p + 2)
                pass2(p)

    # Drop the preamble's GpSimd memsets of unused const tiles: Q7 memsets
    # cost ~us each and gate the post-preamble all-engine barrier, delaying
    # kernel start.  Keep any const a later instruction actually reads.
    used = set()
    for bb in nc.main_func.blocks:
        for inst in bb.instructions:
            if type(inst).__name__ == "InstMemset":
                continue
            for ap in list(inst.ins or []) + list(inst.outs or []):
                ref = getattr(ap, "memref", None)
                if ref and str(ref).startswith("const-"):
                    used.add(str(ref))
    entry = nc.main_func.blocks[0]
    dropped = [
        inst
        for inst in entry.instructions
        if type(inst).__name__ == "InstMemset"
        and inst.outs
        and str(getattr(inst.outs[0], "memref", "")).startswith("const-")
        and str(inst.outs[0].memref) not in used
    ]
    for inst in dropped:
        entry.instructions.remove(inst)

    nc.finalize()
    return nc


def _band_windows_payload(r):
    """Packed nonzero windows of the band-of-ones matrix, [P, A*wmax] f32."""
    win = _band_windows(r)
    wmax = max(n1 - n0 for n0, n1 in win)
    out = np.zeros((P, A * wmax), dtype=np.float32)
    for a, (n0, n1) in enumerate(win):
        for q in range(P):
            i = P * a + q  # absolute row of Bo
            lo = max(n0, i - r, 0)
            hi = min(n1, i + r + 1, W)
            out[q, a * wmax + (lo - n0) : a * wmax + (hi - n0)] = 1.0
    return out


def kernel(x, r):
    import ml_dtypes
    from concourse.bass_utils import run_bass_kernel_spmd

    r = int(r)
    x = np.asarray(x)
    n, c, h, w = x.shape
    assert (h, w) == (H, W) and n == N_CORES, (n, c, h, w)

    key = (r, c)
    if key not in _CACHE:
        _CACHE[key] = _build(r, c)
    nc = _CACHE[key]

    bf16 = ml_dtypes.bfloat16
    xb = np.ascontiguousarray(x.reshape(n, c * H, W)).astype(bf16)
    b = _band_windows_payload(r).astype(bf16)
    in_maps = [{"x": xb[i], "b": b} for i in range(n)]
    res = run_bass_kernel_spmd(nc, in_maps, core_ids=list(range(N_CORES)))
    out = np.stack(
        [np.asarray(res.results[i]["y"]).astype(np.float32).reshape(c, H, W) for i in range(n)]
    )
    return out
